# revision 33
# baseline (speedup 1.0000x reference)
"""Trainium2 Bass kernel for nn_Attention_3710851743764.

Full attention block: qkv proj -> per-head RMSNorm(q,k) -> RoPE -> GQA
attention (16 q heads, 4 kv heads, S=2048, D=128) -> out proj.

Sharding: 8 cores = 2 (batch) x 4 (kv-head groups). Each core computes its
batch's qkv for its group (4 q heads + 1 kv head), full attention for those
heads, and a partial output projection (its 512 wo columns); the host sums
the 4 partials per batch.

All matmul inputs are fp16 (FWL weight loads, 2x DVE rate, half DMA);
PSUM accumulation is fp32. Softmax denominators are accumulated on the
(otherwise idle) DVE in fp16, reduced over key lanes by one tiny PE matmul
per head into PSUM row 0, and normalization is pipelined per head. The PE
is kept continuously busy (HAM stays at max p-state): RMSNorm/RoPE fuse
into the projection stage per token chunk, the second chunk's q-row RoPE
defers into the (scalar-bound) attention stage, and stage E draws its PSUM
from the attention pools' tags so no pool-teardown barrier separates them.

Dataflow is fully "transposed" (features on partitions, tokens on free):
  qkvT[f,t]   = mm(lhsT=wqkvT[d,f], rhs=xT[d,t])            accumulated over d
  ssq[c,t]    = mm(lhsT=esel[:,c,:], rhs=square(qkvT_c))     (RMS factors)
  rot[d',t]   = mm(lhsT=P_rot, rhs=qn)                       (RoPE pair swap)
  scoresT[s,t]= mm(lhsT=kT[:,s-blk], rhs=qT_h)               per 128-s block
  pT          = exp(scoresT)          (no max subtraction: |score|<=sqrt(128))
  attnT[d,t]  = mm(lhsT=v[s-blk,d], rhs=pT)                  accumulated over s
  acc[l,t]    = sum_s pT  (DVE);  denom[t] = mm(lhsT=ones, rhs=acc)
  out[t,o]    = mm(lhsT=attnT_n[f,t-blk], rhs=woT[f,o])      accumulated over f
"""

import sys

sys.path.insert(0, "/opt/trn_rl_repo")

import numpy as np

import concourse.bass as bass
import concourse.tile as tile
from concourse import bacc, mybir
from concourse import bass_utils

F32 = mybir.dt.float32
F16 = mybir.dt.float16
AF = mybir.ActivationFunctionType
OP = mybir.AluOpType

DIM = 2048
N_HEADS = 16
N_KV = 4
HEAD_DIM = 128
B = 2
S = 2048
EPS = float(np.finfo(np.float32).eps)
GQ = N_HEADS // N_KV          # q heads per group = 4
GF = GQ * HEAD_DIM            # group q features = 512
P = 128
KC = DIM // P                 # 16 contraction chunks for projections
TC = 2                        # token chunks of 1024
SC = S // P                   # 16 key chunks of 128
NF = GF + 2 * HEAD_DIM        # 768 qkv features per group
FC = NF // P                  # 6 feature chunks
FCO = (4, 0, 1, 2, 3, 5)      # k first, v last

_CACHED_NC = None


def build_nc():
    """Build the single-core Bass program (same program for all 8 cores)."""
    nc = bacc.Bacc("TRN2", target_bir_lowering=False, debug=False,
                   num_devices=8)

    # DRAM tensors are declared with flat innermost dims so the DMA APs
    # have maximal contiguous runs (4KB+) — [16,128]-shaped APs generate
    # 256B descriptors and run ~4x slower (descriptor-rate-bound).
    xT_d = nc.dram_tensor("xT", [TC, P, KC * 1024], F16,
                          kind="ExternalInput").ap()
    wqkvT_d = nc.dram_tensor("wqkvT", [P, FC, KC * HEAD_DIM], F16,
                             kind="ExternalInput").ap()
    woT_d = nc.dram_tensor("woT", [HEAD_DIM, GQ * DIM], F16,
                           kind="ExternalInput").ap()
    cosT_d = nc.dram_tensor("cosT", [HEAD_DIM, S], F16,
                            kind="ExternalInput").ap()
    sinT_d = nc.dram_tensor("sinT", [HEAD_DIM, S], F16,
                            kind="ExternalInput").ap()
    normw_d = nc.dram_tensor("normw", [P, 2], F32, kind="ExternalInput").ap()
    prot_d = nc.dram_tensor("prot", [P, P], F16, kind="ExternalInput").ap()
    ident_d = nc.dram_tensor("ident", [P, P], F16, kind="ExternalInput").ap()
    esel_d = nc.dram_tensor("esel", [P, 5, 5], F16,
                            kind="ExternalInput").ap()
    out_d = nc.dram_tensor("out", [SC, P, GQ, 512], F16,
                           kind="ExternalOutput").ap()

    with tile.TileContext(nc) as tc:
        with (
            tc.tile_pool(name="consts", bufs=1) as cp,
        ):
            dramp = tc.alloc_tile_pool(name="dram_scratch", bufs=1,
                                       space="DRAM")
            rfac_dr = dramp.tile([5, S], F16, name="rfac_dr")
            rd_dr = [dramp.tile([4, 1024], F16, name=f"rd_dr{i}")
                     for i in range(2)]
            # stage-scoped persistent tensors (manual release for SBUF reuse)
            p1 = tc.alloc_tile_pool(name="p1", bufs=1)   # lives A..rope end
            qkv_raw = p1.tile([P, 5, S], F16, name="qkv_raw")     # 20KB
            vT_sb = p1.tile([P, S], F16, name="vT_sb")            # 4KB

            p2 = tc.alloc_tile_pool(name="p2", bufs=1, side="right")  # A..D
            qk_sb = [p2.tile([P, S], F16, name=f"qk_sb{i}")
                     for i in range(5)]                           # 20KB
            v_sb = p2.tile([P, SC, HEAD_DIM], F16, name="v_sb")   # 4KB

            cos_sb = cp.tile([HEAD_DIM, S], F16, name="cos_sb")
            sin_sb = cp.tile([HEAD_DIM, S], F16, name="sin_sb")
            normw_sb = cp.tile([P, 2], F32, name="normw_sb")
            prot_sb = cp.tile([P, P], F16, name="prot_sb")
            ident_sb = cp.tile([P, P], F16, name="ident_sb")
            esel_sb = cp.tile([P, 5, 5], F16, name="esel_sb")
            eps_sb = cp.tile([P, 1], F32, name="eps_sb")
            zero_sb = cp.tile([P, 1], F32, name="zero_sb")
            nc.vector.memset(eps_sb[:], EPS)
            nc.vector.memset(zero_sb[:], 0.0)

            # ---------------- Stage A: qkv proj + rmsnorm + rope + vT ----
            sr = tc.alloc_tile_pool(name="stR", bufs=3)  # lives into D
            sa = tc.alloc_tile_pool(name="stA", bufs=2)
            wp = tc.alloc_tile_pool(name="wq_pool", bufs=1)
            psA = tc.alloc_tile_pool(name="psA", bufs=2, space="PSUM")
            psSq = tc.alloc_tile_pool(name="psSq", bufs=1, space="PSUM")
            psRT = tc.alloc_tile_pool(name="psRT", bufs=2, space="PSUM")

            wq_sb = wp.tile([P, FC, KC * HEAD_DIM], F16, name="wq_sb")

            def rope_rb(tcc, fc, name="rb", bufs=None):
                """Issue the rms-factor broadcast DMA for one row."""
                tbase = tcc * 1024
                kw = {} if bufs is None else {"bufs": bufs}
                rb = sr.tile([P, 1024], F16, name=name, **kw)
                nc.sync.dma_start(
                    rb[:], rfac_dr[fc:fc + 1, tbase:tbase + 1024]
                    .to_broadcast((P, 1024)))
                return rb

            def emit_rope_fc(tcc, fc, rot_pool, rb=None):
                """RMSNorm + RoPE for one feature row of one chunk."""
                tbase = tcc * 1024
                if rb is None:
                    rb = rope_rb(tcc, fc)
                qn = sr.tile([P, 1024], F16, name="qn")
                wcol = 0 if fc < 4 else 1
                nc.vector.scalar_tensor_tensor(
                    qn[:], qkv_raw[:, fc, tbase:tbase + 1024],
                    normw_sb[:, wcol:wcol + 1], rb[:],
                    op0=OP.mult, op1=OP.mult)
                qc = sr.tile([P, 1024], F16, name="qc")
                nc.gpsimd.tensor_mul(qc[:], qn[:],
                                     cos_sb[:, tbase:tbase + 1024])
                for hf in range(2):
                    tsl = slice(tbase + hf * 512, tbase + hf * 512 + 512)
                    lsl = slice(hf * 512, hf * 512 + 512)
                    rot_ps = rot_pool.tile(
                        [P, 512], F32, name="rot_ps",
                        tag="rt" if rot_pool is psRT else "sp")
                    nc.tensor.matmul(rot_ps[:], prot_sb[:], qn[:, lsl],
                                     start=True, stop=True)
                    rs = sr.tile([P, 512], F16, name="rs")
                    nc.vector.tensor_mul(rs[:], rot_ps[:], sin_sb[:, tsl])
                    nc.vector.tensor_add(qk_sb[fc][:, tsl],
                                         qc[:, lsl], rs[:])

            def emit_vt(tcc):
                """Transpose this chunk's v into [keys, vdim] blocks."""
                for scc in range(tcc * 8, tcc * 8 + 8):
                    vt_ps = psRT.tile([P, P], F16, name="vt_ps", tag="rt")
                    nc.tensor.transpose(
                        vt_ps[:], vT_sb[:, scc * P:(scc + 1) * P],
                        ident_sb[:])
                    nc.vector.tensor_copy(v_sb[:, scc, :], vt_ps[:])

            const_dmas = [
                (esel_sb, esel_d), (normw_sb, normw_d),
                (prot_sb, prot_d), (cos_sb, cosT_d),
                (sin_sb, sinT_d), (ident_sb, ident_d),
            ]
            # x chunks alternate sync/gpsimd in kc order (per-queue transfer
            # throughput, not dispatch, is the early bottleneck; chunk 0
            # uses 1-kc granularity to cut the cold-start latency); wq
            # f-blocks go alone on the scalar queue (fc=4's first — the
            # very first matmul needs it); esel leads the gpsimd queue
            # (first ssq matmul), other consts trail it.
            for tcc in range(TC):
                tbase = tcc * 1024
                xt = sa.tile([P, KC * 1024], F16, name="xt")   # 32KB x2
                if tcc == 0:
                    nc.scalar.dma_start(wq_sb[:, 4], wqkvT_d[:, 4])
                    nc.gpsimd.dma_start(esel_sb[:], esel_d)
                    const_dmas.pop(0)
                step = 1 if tcc == 0 else 2
                for i, kc1 in enumerate(range(0, KC, step)):
                    q = nc.sync if i % 2 == 0 else nc.gpsimd
                    csl = slice(kc1 * 1024, (kc1 + step) * 1024)
                    q.dma_start(xt[:, csl], xT_d[tcc, :, csl])
                if tcc == 0:
                    for f in (0, 1, 2, 3, 5):
                        nc.scalar.dma_start(wq_sb[:, f], wqkvT_d[:, f])
                    for dst, src in const_dmas:
                        nc.gpsimd.dma_start(dst[:], src)
                    const_dmas = []
                ssq_ps = psSq.tile([5, 1024], F32, name="ssq_ps")
                for fi, fc in enumerate(FCO):
                    ps = psA.tile([P, 1024], F32, name="qkv_ps")
                    for kc in range(KC):
                        for hf in range(2):
                            hsl = slice(hf * 512, hf * 512 + 512)
                            nc.tensor.matmul(
                                ps[:, hsl],
                                wq_sb[:, fc, kc * P:(kc + 1) * P],
                                xt[:, kc * 1024 + hf * 512:
                                   kc * 1024 + hf * 512 + 512],
                                start=(kc == 0), stop=(kc == KC - 1))
                    if fc != 5:
                        sq = sa.tile([P, 1024], F16, name="sq")
                        nc.scalar.activation(sq[:], ps[:], AF.Square,
                                             bias=zero_sb[:])
                        for hf in range(2):
                            hsl = slice(hf * 512, hf * 512 + 512)
                            nc.tensor.matmul(
                                ssq_ps[:, hsl],
                                esel_sb[:, fc, :],
                                sq[:, hsl],
                                start=(fc == 4), stop=(fc == 3),
                                skip_group_check=True)
                        nc.vector.tensor_copy(
                            qkv_raw[:, fc, tbase:tbase + 1024], ps[:])
                        # per-row rms factor -> DRAM (for broadcast).
                        # All 5 rows are processed each time (same cost,
                        # free-dim bound; engines need base partition 0)
                        # but only the just-completed row fc is shipped.
                        std = sa.tile([5, 1024], F32, name="std")
                        nc.scalar.activation(std[:], ssq_ps[:], AF.Sqrt,
                                             scale=1.0 / HEAD_DIM,
                                             bias=eps_sb[0:5, :])
                        rfacf = sa.tile([5, 1024], F32, name="rfacf")
                        nc.vector.reciprocal_approx_fast(rfacf[:], std[:])
                        rfac16 = sa.tile([5, 1024], F16, name="rfac16")
                        nc.vector.tensor_copy(rfac16[:], rfacf[:])
                        nc.gpsimd.dma_start(
                            rfac_dr[fc:fc + 1, tbase:tbase + 1024],
                            rfac16[fc:fc + 1, :])
                    else:
                        nc.vector.tensor_copy(
                            vT_sb[:, tbase:tbase + 1024], ps[:])
                    # overlap previous chunk's rope under this chunk
                    if tcc == 1 and fi < 5:
                        emit_rope_fc(0, FCO[fi], psRT)
                    if tcc == 1 and fi == 5:
                        emit_vt(0)
            # chunk 1: k row + v transposes now; q rows defer into stage D
            emit_rope_fc(TC - 1, 4, psRT)
            emit_vt(TC - 1)

            psRT.release()
            psSq.release()
            psA.release()
            wp.release()
            sa.release()

            # ---------------- Stage D: attention + Stage E: out proj -----
            p3 = tc.alloc_tile_pool(name="p3", bufs=1)
            atn_n = [p3.tile([P, GQ, 1024], F16, name=f"atn_n{i}")
                     for i in range(2)]                           # 16KB
            woT_sb = p3.tile([P, GQ * DIM], F16, name="woT_sb")   # 16KB
            nc.scalar.dma_start(woT_sb[:], woT_d)
            with (
                tc.tile_pool(name="stD", bufs=2) as sd,
                tc.tile_pool(name="atr", bufs=2) as atr,
                tc.tile_pool(name="accp", bufs=2) as accp,
                tc.tile_pool(name="ptp", bufs=4) as ptp,
                tc.tile_pool(name="psS", bufs=3, space="PSUM") as psS,
                tc.tile_pool(name="psPV", bufs=1, space="PSUM") as psPV,
            ):
                def emit_pv(pv_ps, pt, scc):
                    for hf in range(2):
                        hs = slice(hf * 512, hf * 512 + 512)
                        nc.tensor.matmul(
                            pv_ps[:, hs], v_sb[:, scc, :], pt[:, hs],
                            start=(scc == 0), stop=(scc == SC - 1))

                def emit_dn(pair, h, acc, atn_raw):
                    """Denominator reduce -> reciprocal -> broadcast ->
                    normalize for one head (emitted one head late so the
                    PE never waits on the DVE's last acc add)."""
                    dnt = psS.tile([1, 1024], F32, name="dnt", tag="sp")
                    for hf in range(2):
                        hs = slice(hf * 512, hf * 512 + 512)
                        nc.tensor.matmul(
                            dnt[0:1, hs], esel_sb[:, 0, 0:1],
                            acc[:, hs], start=True, stop=True)
                    rdf = sd.tile([1, 1024], F32, name="rdf")
                    nc.vector.reciprocal_approx_fast(rdf[:], dnt[0:1, :])
                    rd16 = sd.tile([1, 1024], F16, name="rd16")
                    nc.vector.tensor_copy(rd16[:], rdf[:])
                    nc.gpsimd.dma_start(rd_dr[pair][h:h + 1, :], rd16[:])
                    rbh = sd.tile([P, 1024], F16, name="rbh")
                    nc.sync.dma_start(
                        rbh[:],
                        rd_dr[pair][h:h + 1, :].to_broadcast((P, 1024)))
                    nc.vector.tensor_mul(atn_n[pair][:, h, :],
                                         atn_raw[:], rbh[:])

                rope_defer = [(TC - 1, fc) for fc in (0, 1, 2, 3)]
                rope_rbs = [rope_rb(t, f, name="rbd", bufs=4)
                            for t, f in rope_defer]
                pend_dn = None
                for pair in range(2):
                    po = pair * 1024
                    for h in range(GQ):
                        pv_ps = psPV.tile([P, 1024], F32, name="pv_ps")
                        acc = accp.tile([P, 1024], F16, name="acc")
                        prev = None
                        for scc in range(SC):
                            # deferred chunk-1 q-row rope, spread over the
                            # first (scalar-bound) head's iterations
                            if pair == 0 and h == 0 and scc % 3 == 1 \
                                    and rope_defer:
                                emit_rope_fc(*rope_defer.pop(0), psS,
                                             rb=rope_rbs.pop(0))
                            ksl = qk_sb[4][:, scc * P:(scc + 1) * P]
                            sp = psS.tile([P, 1024], F32, name="sp",
                                          tag="sp")
                            for hf in range(2):
                                hs = slice(hf * 512, hf * 512 + 512)
                                nc.tensor.matmul(
                                    sp[:, hs], ksl,
                                    qk_sb[h][:, po + hf * 512:
                                              po + hf * 512 + 512],
                                    start=True, stop=True)
                            pt = ptp.tile([P, 1024], F16, name="pt")
                            nc.scalar.activation(pt[:], sp[:], AF.Exp,
                                                 bias=zero_sb[:])
                            # denominator partial sums on the (idle) DVE
                            if scc == 0:
                                nc.vector.tensor_copy(acc[:], pt[:])
                            else:
                                nc.vector.tensor_add(acc[:], acc[:], pt[:])
                            if prev is not None:
                                emit_pv(pv_ps, *prev)
                            prev = (pt, scc)
                            if scc == 2 and pend_dn is not None:
                                emit_dn(*pend_dn)
                                pend_dn = None
                        emit_pv(pv_ps, *prev)
                        atn_raw = atr.tile([P, 1024], F16, name="atn_raw")
                        nc.vector.tensor_copy(atn_raw[:], pv_ps[:])
                        pend_dn = (pair, h, acc, atn_raw)
                emit_dn(*pend_dn)

                # ---------- Stage E (same pools: no teardown barrier) ----
                with tc.tile_pool(name="stE", bufs=4) as se:
                    for tcc in range(SC):
                        pr = tcc // 8
                        tloc = (tcc % 8) * P
                        o2 = [psS.tile([P, 2, 512], F32, name="out_ps",
                                       tag="sp") for _ in range(2)]
                        for h in range(GQ):
                            lhs = atn_n[pr][:, h, tloc:tloc + P]
                            for oc in range(4):
                                nc.tensor.matmul(
                                    o2[oc // 2][:, oc % 2, :], lhs,
                                    woT_sb[:, h * DIM + oc * 512:
                                           h * DIM + oc * 512 + 512],
                                    start=(h == 0), stop=(h == GQ - 1),
                                    skip_group_check=True)
                        ob = se.tile([P, GQ, 512], F16, name="ob")
                        for oc in range(4):
                            if oc % 2 == 0:
                                nc.vector.tensor_copy(
                                    ob[:, oc, :], o2[oc // 2][:, oc % 2, :])
                            else:
                                nc.scalar.activation(
                                    ob[:, oc, :], o2[oc // 2][:, oc % 2, :],
                                    AF.Copy)
                        oq = (nc.sync, nc.gpsimd, nc.scalar)[tcc % 3]
                        oq.dma_start(out_d[tcc], ob[:])
            p2.release()
            p3.release()
            sr.release()
            p1.release()

    nc.compile()
    return nc


def make_in_maps(x, wqkv, wo, q_norm_w, k_norm_w, freqs_cos, freqs_sin):
    """Build the 8 per-core input maps. Core c = b*4 + g."""
    x = np.asarray(x, np.float32)
    wqkv = np.asarray(wqkv, np.float32)
    wo = np.asarray(wo, np.float32)
    q_norm_w = np.asarray(q_norm_w, np.float32)
    k_norm_w = np.asarray(k_norm_w, np.float32)
    cosT = np.ascontiguousarray(
        np.asarray(freqs_cos, np.float32)[:, 0, :].T).astype(np.float16)
    sinT = np.ascontiguousarray(
        np.asarray(freqs_sin, np.float32)[:, 0, :].T).astype(np.float16)

    normw = np.empty((P, 2), np.float32)
    normw[:, 0] = q_norm_w * np.float32(1.0 / np.sqrt(HEAD_DIM))
    normw[:, 1] = k_norm_w

    prot = np.zeros((P, P), np.float16)
    prot[np.arange(1, P, 2), np.arange(0, P, 2)] = -1.0
    prot[np.arange(0, P, 2), np.arange(1, P, 2)] = 1.0
    ident = np.eye(P, dtype=np.float16)
    esel = np.zeros((P, 5, 5), np.float16)
    for c in range(5):
        esel[:, c, c] = 1.0

    q_size = N_HEADS * HEAD_DIM
    kv_size = N_KV * HEAD_DIM
    in_maps = []
    for b in range(B):
        # [tc, p, kc*1024+u]: xT[kc*128+p, tc*1024+u] pre-tiled, 4KB runs
        xT = np.ascontiguousarray(
            x[b].reshape(TC, 1024, KC, P).transpose(0, 3, 2, 1)
        ).astype(np.float16).reshape(TC, P, KC * 1024)
        for g in range(N_KV):
            wq = wqkv[g * GF:(g + 1) * GF]
            wk = wqkv[q_size + g * HEAD_DIM:q_size + (g + 1) * HEAD_DIM]
            wv = wqkv[q_size + kv_size + g * HEAD_DIM:
                      q_size + kv_size + (g + 1) * HEAD_DIM]
            # fc-major: [p, f, kc*128+j] = W[f*128+j, kc*128+p]
            wqkvT = np.ascontiguousarray(
                np.concatenate([wq, wk, wv], axis=0).T
                .reshape(KC, P, FC, HEAD_DIM).transpose(1, 2, 0, 3)
            ).astype(np.float16).reshape(P, FC, KC * HEAD_DIM)
            woT = np.ascontiguousarray(
                wo[:, g * GF:(g + 1) * GF].T.reshape(GQ, HEAD_DIM, DIM)
                .transpose(1, 0, 2)).astype(np.float16) \
                .reshape(HEAD_DIM, GQ * DIM)
            in_maps.append({
                "xT": xT, "wqkvT": wqkvT, "woT": woT,
                "cosT": cosT, "sinT": sinT, "normw": normw,
                "prot": prot, "ident": ident, "esel": esel,
            })
    return in_maps


def run(in_maps, trace=False):
    global _CACHED_NC
    if _CACHED_NC is None:
        _CACHED_NC = build_nc()
    return bass_utils.run_bass_kernel_spmd(
        _CACHED_NC, in_maps, core_ids=list(range(8)), trace=trace)


def kernel(x, wqkv, wo, q_norm_w, k_norm_w, freqs_cos, freqs_sin):
    in_maps = make_in_maps(x, wqkv, wo, q_norm_w, k_norm_w,
                           freqs_cos, freqs_sin)
    res = run(in_maps, trace=False)
    out = np.zeros((B, S, DIM), np.float32)
    for b in range(B):
        for g in range(N_KV):
            o = res.results[b * N_KV + g]["out"]    # [SC, P, GQ, 512]
            out[b] += o.reshape(S, DIM).astype(np.float32)
    return out
